# revision 1
# baseline (speedup 1.0000x reference)
"""LGnet (LSTM-style gated recurrent net) Trainium2 Bass kernel.

B=64, T=256, D=512, H=512, COMB=1536. Data-parallel over batch across 8
NeuronCores (B_local=8 per core).

Decomposition: comb @ W = xt' @ W[0:512] + h' @ W[512:1024] + m @ W[1024:1536].
Only the h' term is recurrent; everything else (xt', delta_x, delta_h, the
xt'/m gate contributions "a") is precomputed for all (t, b) with large
matmuls.  The sequential phase then does one [8,512] @ [512,2048] bf16
matmul per step in weight-stationary transposed-gate layout so the
elementwise chain runs on full 128-partition tiles.

The wire to the device is slow (~60 MB/s through the tunnel, half-duplex),
so the host interface is optimized for bytes:
  - the xt' elementwise chain (delta_x mixing) runs on the host in f32,
    chunk-pipelined under the transfer, so only 3 derived fields ship:
    xt' and m as bf16 (32 MB), dl as fp8-e4m3 (8 MB; it only feeds the
    delta_h matmul, where fp8 input error is negligible).  The kernel
    transposes them on-device with xbar DMA-transpose (dl is upcast to
    bf16 through a DRAM bounce first, since the xbar needs 2-byte types).
  - weights ship bf16 once and stay device-resident across calls
    (re-verified by content hash each call).
  - output returns as bf16 [HC,128,T,BL] (16 MB total).
  - the jitted executor is cached so warm calls skip re-trace/compile;
    donated output buffers are zero-filled on-device each call.

Per-core layouts, R = T*B_l = 2048, row index r = t*8 + b:
  xb   : [8, 2, 256, 512] bf16 input (b, field, t, d), fields = xt', m
  dl8  : [8, 256, 512] fp8-e4m3 input
  wg   : [8, 128, 2048] bf16 non-recurrent gate weights (K = xt rows 0:512
         then m rows 1024:1536), N = [i|f|o|c] * 512
  wh   : [4, 128, 2048] bf16 recurrent rows 512:1024
  ghw  : [4, 128, 512] bf16
  pvec : [128, 28] f32 per-partition params: ghb(4) at 8:12, bg(16) at 12:28
  hT_out : [4, 128, 256, 8] bf16 h^T staged output [hc, p, t, b]
"""

import os
import time
import hashlib
import numpy as np
import ml_dtypes

B, T, D, H = 64, 256, 512, 512
NCORES = 8
BL = B // NCORES          # 8 per-core batch
R = T * BL                # 2048 rows
DC, HC, GC = 4, 4, 16     # 128-chunks of D, H, and 4*H gate cols
G = 4 * H                 # 2048 gate columns
SS = 16                   # recurrence steps per staging block

BF16 = ml_dtypes.bfloat16
F8E4 = ml_dtypes.float8_e4m3
# uint8 output decode offset: the HW activation rounds on float->uint8
# (CoreSim truncates — hardware measured as round-to-nearest), so the
# encode q = round(127*h + 128) decodes as (q - 128)/127.
_DEC_OFF = np.float32(128.0)
_CACHE = {}
_PROF = bool(int(os.environ.get("LG_PROF", "0")))


def _t(msg, t0):
    if _PROF:
        print(f"[lgnet] {msg}: {time.time() - t0:.3f}s", flush=True)
    return time.time()


def _build():
    import concourse.bacc as bacc
    import concourse.tile as tile
    import concourse.mybir as mybir

    f32 = mybir.dt.float32
    bf16 = mybir.dt.bfloat16
    AF = mybir.ActivationFunctionType

    nc = bacc.Bacc("TRN2", target_bir_lowering=False, debug=False)

    f8 = mybir.dt.float8e4
    u8 = mybir.dt.uint8
    dt_in = {}
    for name, shape, dt in [
        ("xb", (BL, 2, 256, 512), bf16),
        ("dl8", (BL, 256, 512), f8),
        ("wg", (8, 128, G), bf16), ("wh", (HC, 128, G), bf16),
        ("ghw", (DC, 128, D), bf16), ("pvec", (128, 28), f32),
    ]:
        dt_in[name] = nc.dram_tensor(name, list(shape), dt, kind="ExternalInput").ap()
    # output as offset-uint8: q = 127*h + 128 converted to uint8; |h| < 1
    # so q never clips.  Host decodes with _DEC_OFF.
    out_ap = nc.dram_tensor("hT_out", [HC, 128, T, BL], u8, kind="ExternalOutput").ap()

    with tile.TileContext(nc) as tc:
        with (
            tc.tile_pool(name="const", bufs=1) as cp,
            tc.tile_pool(name="stage", bufs=4) as sp,
            tc.tile_pool(name="work", bufs=2) as wp,
            tc.tile_pool(name="ablk", bufs=2) as abp,
            tc.tile_pool(name="hstg", bufs=2) as stp,
            tc.tile_pool(name="psum", bufs=4, space="PSUM") as pp,
            tc.tile_pool(name="psum3", bufs=2, space="PSUM") as pp3,
            tc.tile_pool(name="dram", bufs=1, space="DRAM") as dp,
        ):
            # ---- resident tiles ----
            pvec = cp.tile([128, 28], f32)
            nc.sync.dma_start(pvec[:], dt_in["pvec"][:])
            whs = cp.tile([128, HC, G], bf16)
            for kc in range(HC):
                nc.sync.dma_start(whs[:, kc, :], dt_in["wh"][kc])
            wgs = cp.tile([128, 8, G], bf16)
            for kc in range(8):
                nc.sync.dma_start(wgs[:, kc, :], dt_in["wg"][kc])
            ghwt = cp.tile([128, DC, D], bf16)
            for kc in range(DC):
                nc.sync.dma_start(ghwt[:, kc, :], dt_in["ghw"][kc])

            xtp = cp.tile([128, DC, R], bf16)     # xt' (host-computed)
            mres = cp.tile([128, DC, R], bf16)    # m
            dlr = cp.tile([128, DC, R], bf16)     # dl
            dht = cp.tile([128, HC, R], bf16)     # delta_h^T
            aT = dp.tile([GC, 128, R], bf16)      # gate preactivation staging
            dlbf = dp.tile([BL, 256, 512], bf16)  # dl upcast staging

            def interleave(tile_, b):
                # [:, :, b::8] view of a [128, DC, R] tile
                return tile_.rearrange("p c (t b) -> p c t b", b=BL)[:, :, :, b]

            # ---- phase 0: transpose x fields (xt', m, dl) ----
            # dl ships fp8 (it only feeds the delta_h matmul); the xbar
            # transpose needs a 2-byte dtype, so upcast to bf16 through a
            # DRAM staging bounce first.
            for b in range(BL):
                for th in range(2):
                    l8 = sp.tile([128, 512], f8, tag="l8")
                    nc.sync.dma_start(
                        l8[:], dt_in["dl8"][b][th * 128:(th + 1) * 128, :])
                    u16 = sp.tile([128, 512], bf16, tag="u16")
                    nc.gpsimd.tensor_copy(u16[:], l8[:])
                    nc.sync.dma_start(
                        dlbf[b][th * 128:(th + 1) * 128, :], u16[:])
            for fi, dst in enumerate([xtp, mres, dlr]):
                for b in range(BL):
                    stg = sp.tile([128, DC, 256], bf16, tag="stg")
                    src = (dt_in["xb"][b, fi] if fi < 2 else dlbf[b])
                    nc.sync.dma_start_transpose(stg[:], src)
                    eng = nc.vector if fi % 2 == 0 else nc.gpsimd
                    eng.tensor_copy(interleave(dst, b), stg[:])

            NB = R // 512  # 4 blocks of 512 rows

            # ---- phase 1b: delta_h = exp(-relu(dl @ ghW + ghb)) ----
            for nb in range(NB):
                sl = slice(nb * 512, (nb + 1) * 512)
                for mt in range(HC):
                    ps = pp.tile([128, 512], f32, tag="mmps")
                    for kc in range(DC):
                        nc.tensor.matmul(
                            ps[:],
                            ghwt[:, kc, mt * 128:(mt + 1) * 128],
                            dlr[:, kc, sl],
                            start=(kc == 0), stop=(kc == DC - 1))
                    t1 = wp.tile([128, 512], f32, tag="dht1")
                    nc.scalar.activation(t1[:], ps[:], AF.Relu,
                                         bias=pvec[:, 8 + mt:9 + mt], scale=1.0)
                    nc.scalar.activation(dht[:, mt, sl], t1[:], AF.Exp, scale=-1.0)

            # ---- phase 2: a = xt'@Wx + m@Wm + bias  -> aT dram ----
            for gc in range(GC):
                for nb in range(NB):
                    sl = slice(nb * 512, (nb + 1) * 512)
                    ps = pp.tile([128, 512], f32, tag="mmps")
                    for kc in range(8):
                        rhs = xtp[:, kc, sl] if kc < DC else mres[:, kc - DC, sl]
                        nc.tensor.matmul(ps[:], wgs[:, kc, gc * 128:(gc + 1) * 128],
                                         rhs, start=(kc == 0), stop=(kc == 7))
                    ao = wp.tile([128, 512], bf16, tag="ao")
                    nc.scalar.activation(ao[:], ps[:], AF.Identity,
                                         bias=pvec[:, 12 + gc:13 + gc], scale=1.0)
                    nc.sync.dma_start(aT[gc][:, sl], ao[:])

            # ---- phase 3: recurrence ----
            c_st = cp.tile([128, HC, BL], f32)
            hbf = cp.tile([128, HC, BL], bf16)
            nc.vector.memset(c_st[:], 0.0)
            nc.vector.memset(hbf[:], 0.0)

            for blk in range(T // SS):
                t0 = blk * SS
                ab = abp.tile([128, GC, SS * BL], bf16, tag="ab")
                for gc in range(GC):
                    nc.sync.dma_start(ab[:, gc, :], aT[gc][:, t0 * BL:(t0 + SS) * BL])
                hst = stp.tile([128, SS, HC, BL], bf16, tag="hst")
                for s in range(SS):
                    t = t0 + s
                    gps = pp3.tile([128, GC, BL], f32, tag="gps")
                    for gc in range(GC):
                        for kc in range(HC):
                            nc.tensor.matmul(
                                gps[:, gc, :],
                                whs[:, kc, gc * 128:(gc + 1) * 128],
                                hbf[:, kc, :],
                                start=(kc == 0), stop=(kc == HC - 1))
                    g = wp.tile([128, GC, BL], f32, tag="g")
                    nc.vector.tensor_add(g[:], gps[:],
                                         ab[:, :, s * BL:(s + 1) * BL])
                    ga = wp.tile([128, GC, BL], f32, tag="ga")
                    nc.scalar.activation(ga[:, 0:12, :], g[:, 0:12, :], AF.Sigmoid)
                    nc.scalar.activation(ga[:, 12:16, :], g[:, 12:16, :], AF.Tanh)
                    tn = min(t + 1, T - 1)
                    odh = wp.tile([128, HC, BL], f32, tag="odh")
                    nc.vector.tensor_mul(odh[:], ga[:, 8:12, :],
                                         dht[:, :, tn * BL:(tn + 1) * BL])
                    tmp = wp.tile([128, HC, BL], f32, tag="tmp")
                    nc.vector.tensor_mul(tmp[:], ga[:, 0:4, :], ga[:, 12:16, :])
                    nc.vector.tensor_mul(c_st[:], c_st[:], ga[:, 4:8, :])
                    nc.vector.tensor_add(c_st[:], c_st[:], tmp[:])
                    th = wp.tile([128, HC, BL], f32, tag="th")
                    nc.scalar.activation(th[:], c_st[:], AF.Tanh)
                    nc.vector.tensor_mul(hst[:, s, :, :], ga[:, 8:12, :], th[:])
                    nc.vector.tensor_mul(hbf[:], odh[:], th[:])
                h8 = stp.tile([128, SS, HC, BL], u8, tag="h8")
                nc.scalar.activation(h8[:], hst[:], AF.Identity,
                                     bias=pvec[:, 0:1], scale=127.0)
                for hc in range(HC):
                    nc.sync.dma_start(out_ap[hc][:, t0:t0 + SS, :],
                                      h8[:, :, hc, :])

    nc.compile()
    return nc


def _prep_weights(Wi, bi, Wf, bf, Wo, bo, Wc, bc, gh_W, gh_b, **_unused):
    f32 = np.float32
    Wfull = np.concatenate([Wi, Wf, Wo, Wc], axis=1).astype(f32)   # [1536, 2048]
    bfull = np.concatenate([bi, bf, bo, bc]).astype(f32)           # [2048]
    wg = np.concatenate([Wfull[0:512], Wfull[1024:1536]], axis=0
                        ).reshape(8, 128, G).astype(BF16)
    wh = Wfull[512:1024].reshape(HC, 128, G).astype(BF16)
    ghw = gh_W.astype(f32).reshape(DC, 128, D).astype(BF16)
    pvec = np.zeros((128, 28), f32)
    pvec[:, 0] = 128.0                  # uint8 output encode offset
    pvec[:, 8:12] = gh_b.astype(f32).reshape(4, 128).T
    pvec[:, 12:28] = bfull.reshape(16, 128).T
    return {"wg": wg, "wh": wh, "ghw": ghw, "pvec": pvec}


def _get_exec(nc):
    """Cached jitted SPMD executor for the compiled Bass module.

    Mirrors concourse.bass_utils.run_bass_kernel_spmd's axon path
    (bass2jax.run_bass_via_pjrt) but keeps the jitted function and the
    device-resident weight buffers alive across calls, and fills the
    donated output buffers with on-device zeros instead of shipping them
    over the wire every call.
    """
    if "exec" in _CACHE:
        return _CACHE["exec"]

    import jax
    import jax.numpy as jnp
    import concourse.mybir as mybir
    from jax.sharding import Mesh, PartitionSpec, NamedSharding
    from jax.experimental.shard_map import shard_map
    from concourse.bass2jax import (
        _bass_exec_p, install_neuronx_cc_hook, partition_id_tensor)

    install_neuronx_cc_hook()

    partition_name = (nc.partition_id_tensor.name
                      if nc.partition_id_tensor else None)
    in_names, out_names, out_avals, out_shapes = [], [], [], []
    for alloc in nc.m.functions[0].allocations:
        if not isinstance(alloc, mybir.MemoryLocationSet):
            continue
        name = alloc.memorylocations[0].name
        if alloc.kind == "ExternalInput":
            if name != partition_name:
                in_names.append(name)
        elif alloc.kind == "ExternalOutput":
            shape = tuple(alloc.tensor_shape)
            dtype = mybir.dt.np(alloc.dtype)
            out_names.append(name)
            out_avals.append(jax.core.ShapedArray(shape, dtype))
            out_shapes.append((shape, dtype))
    n_params = len(in_names)
    n_outs = len(out_names)
    all_in_names = in_names + out_names
    if partition_name is not None:
        all_in_names.append(partition_name)
    donate = tuple(range(n_params, n_params + n_outs))

    def _body(*args):
        operands = list(args)
        if partition_name is not None:
            operands.append(partition_id_tensor())
        outs = _bass_exec_p.bind(
            *operands,
            out_avals=tuple(out_avals),
            in_names=tuple(all_in_names),
            out_names=tuple(out_names),
            lowering_input_output_aliases=(),
            sim_require_finite=True,
            sim_require_nnan=True,
            nc=nc,
        )
        return tuple(outs)

    devices = jax.devices()[:NCORES]
    mesh = Mesh(np.asarray(devices), ("core",))
    sharding = NamedSharding(mesh, PartitionSpec("core"))
    in_specs = (PartitionSpec("core"),) * (n_params + n_outs)
    out_specs = (PartitionSpec("core"),) * n_outs
    jitfn = jax.jit(
        shard_map(_body, mesh=mesh, in_specs=in_specs, out_specs=out_specs,
                  check_rep=False),
        donate_argnums=donate, keep_unused=True)

    def _zeros():
        return tuple(
            jnp.zeros((NCORES * s[0], *s[1:]), d) for s, d in out_shapes)
    zeros_fn = jax.jit(_zeros, out_shardings=(sharding,) * n_outs)

    exec_state = {
        "jitfn": jitfn, "zeros_fn": zeros_fn, "sharding": sharding,
        "in_names": in_names, "out_names": out_names,
        "jax": jax, "devices": devices,
    }
    _CACHE["exec"] = exec_state
    return exec_state


def kernel(**inputs):
    t0 = time.time()
    if "nc" not in _CACHE:
        _CACHE["nc"] = _build()
        t0 = _t("build+compile", t0)
    nc = _CACHE["nc"]
    ex = _get_exec(nc)
    jax = ex["jax"]

    # The xt' elementwise chain runs on the host in f32 (it is cheap and
    # pipelines under the slow wire), so only 3 fields ship: xt', m, dl.
    # Chunk per core so compute/cast of chunk i+1 overlaps the async
    # transfer of chunk i.
    x = inputs["x"]
    Xm = np.asarray(inputs["X_mean"], np.float32)
    gxw = np.asarray(inputs["gx_w"], np.float32)
    gxb = np.asarray(inputs["gx_b"], np.float32)
    devices = ex["devices"]

    # weight re-verification hash runs on a thread (hashlib releases the
    # GIL) and the donated on-device zero buffers dispatch up front, so
    # both overlap the x transfer instead of extending the critical path
    import threading
    wkeys = [k for k in sorted(inputs) if k != "x"]
    hres = {}

    def _hash_weights():
        h = hashlib.sha1()
        for k in wkeys:
            h.update(np.ascontiguousarray(inputs[k]))
        hres["h"] = h.hexdigest()

    hth = threading.Thread(target=_hash_weights)
    hth.start()
    zeros = ex["zeros_fn"]()

    shards, shards8 = [], []
    for c in range(NCORES):
        xc = np.asarray(x[c * BL:(c + 1) * BL], np.float32)
        xt, xl, m, dl = xc[:, 0], xc[:, 1], xc[:, 2], xc[:, 3]
        dx = np.exp(-np.maximum(0.0, dl * gxw + gxb))
        chunk = np.empty((BL, 2, 256, 512), BF16)
        chunk[:, 0] = m * xt + (1.0 - m) * (dx * xl + (1.0 - dx) * Xm)
        chunk[:, 1] = m
        shards.append(jax.device_put(chunk, devices[c]))
        shards8.append(jax.device_put(dl.astype(F8E4), devices[c]))
    xb_arr = jax.make_array_from_single_device_arrays(
        (B, 2, 256, 512), ex["sharding"], shards)
    dl_arr = jax.make_array_from_single_device_arrays(
        (B, 256, 512), ex["sharding"], shards8)
    t0 = _t("x host-compute+put", t0)

    hth.join()
    hsh = hres["h"]
    if _CACHE.get("whash") != hsh:
        wmaps = _prep_weights(**{k: inputs[k] for k in wkeys})
        dev_w = {}
        for name, arr in wmaps.items():
            rep = np.concatenate([arr] * NCORES, axis=0)
            dev_w[name] = jax.device_put(rep, ex["sharding"])
        for v in dev_w.values():
            v.block_until_ready()
        _CACHE["dev_w"] = dev_w
        _CACHE["whash"] = hsh
    t0 = _t("weights", t0)

    per_call = {"xb": xb_arr, "dl8": dl_arr}
    args = []
    for name in ex["in_names"]:
        args.append(per_call.get(name) if name in per_call
                    else _CACHE["dev_w"][name])
    args.extend(zeros)
    outs = ex["jitfn"](*args)
    out = outs[0]                             # [8*HC, 128, T, BL] bf16
    out.copy_to_host_async()
    t0 = _t("dispatch", t0)

    # assemble per-shard so host transform overlaps the remaining D2H;
    # decode the offset-uint8 encoding (device truncates 127*h + 128)
    res = np.empty((B, T, H), np.float32)
    for shard in sorted(out.addressable_shards, key=lambda s: s.index[0].start):
        c = shard.index[0].start // HC
        blk = np.asarray(shard.data)          # [HC, 128, T, BL] uint8
        dec = blk.transpose(3, 2, 0, 1).reshape(BL, T, H).astype(np.float32)
        dec -= _DEC_OFF
        dec *= np.float32(1.0 / 127.0)
        res[c * BL:(c + 1) * BL] = dec
    _t("D2H+assemble", t0)
    return res

